# revision 6
# baseline (speedup 1.0000x reference)
"""ChebNet (K=4, 2 ChebConv layers + relu + log_softmax) on 8 trn2 NeuronCores.

Strategy (graph/data parallel, dense-ified SpMM on the TensorEngine):
  - prop(h) = A @ h with A = -diag(dis) @ Cnt @ diag(dis); Cnt entries are
    small ints, exact in fp8e4m3. Nodes sharded 8 ways by destination; each
    core keeps its Cnt^T shard [src=N_pad, dst=DLOC] SBUF-resident in fp8
    (~12.6 MB) and computes its 1/8 of prop outputs as fp8 DoubleRow matmuls
    (256-src-node contraction per instruction, 157 TF/s).
  - Between propagation steps the per-core [DLOC, F] fp8 shards are exchanged
    with 8-core AllGathers (chunked 3-ways to pipeline against compute).
  - Schedule (from trace analysis of the 326us baseline):
      * a tiny dummy AllGather fires at t~0 so the collective path's ~60us
        cold init runs concurrently with the A-matrix HBM load + k=1.
      * layer-0 k=1 matmuls are emitted in A-arrival order (ag=8 load
        groups) so the PE streams right behind the 13 MB HBM load.
      * steady steps emit chunk-0's full contraction first (early AG
        trigger), then the remaining chunks gci-major so matmuls unblock
        in AG-chunk arrival order. PSUM 'pp' bufs=3 keeps 3 accumulations
        live.
      * warm matmuls are anchored via a data dep (rhs=tbf) so they fill AG
        waits instead of being hoisted to t=0 by the scheduler.
      * tail: tbf/bias work on DVE, relu computed once, log_softmax batched
        with the Exp ACT-table primed early (ACT_TABLE_LOAD is a 1.28us
        stall), single output DMA.
"""

import sys

sys.path.insert(0, "/opt/trn_rl_repo")

import numpy as np
import ml_dtypes

import concourse.bacc as bacc
import concourse.mybir as mybir
import concourse.tile as tile
from concourse.bass_utils import run_bass_kernel_spmd
from concourse.masks import make_identity

F32 = mybir.dt.float32
BF16 = mybir.dt.bfloat16
F8E4 = mybir.dt.float8e4

NCORES = 8
P = 128

N = 10000
F_IN = 128
HID = 128
C_OUT = 16
K_ORD = 4


class Geom:
    def __init__(self, n_nodes, tiles_per_core, f_in=F_IN, hid=HID, c_out=C_OUT,
                 k_ord=K_ORD):
        self.n = n_nodes
        self.tpc = tiles_per_core          # src tiles per core (DLOC/128)
        self.dloc = tiles_per_core * P     # nodes per core (padded)
        self.npad = self.dloc * NCORES     # padded node count
        self.nt = self.npad // P           # total src tiles
        self.f = f_in
        self.hid = hid
        self.c = c_out
        self.k = k_ord
        assert self.npad >= n_nodes
        assert f_in == P and hid == P
        # psum chunking of the dloc free dim (max 512 fp32 per bank).
        self.chunks = []
        off = 0
        while off < self.dloc:
            sz = 256 if (off == 0 and self.dloc > 512) else \
                min(512, self.dloc - off)
            self.chunks.append((off, sz))
            off += sz
        self.ctiles = [(off // P, (off + sz) // P) for off, sz in self.chunks]
        assert all((t1 - t0) % 2 == 0 for t0, t1 in self.ctiles)
        # a-tile groups (DMA granularity): small so k=1 streams behind HBM
        self.ag = next(a for a in (8, 16, 40) if self.nt % a == 0)
        self.n_agrp = self.nt // self.ag
        self.n_ggrp = NCORES

    def gci_of_tile(self, t):
        for gci, (t0, t1) in enumerate(self.ctiles):
            if t0 <= t < t1:
                return gci
        raise AssertionError(t)


FULL = Geom(N, 10)  # 1280 nodes/core, npad=10240, 80 src tiles

N_WARM_STEP = 6


def build_nc(g: Geom):
    nc = bacc.Bacc("TRN2", target_bir_lowering=False, debug=False,
                   num_devices=NCORES)

    # ---- kernel I/O ----------------------------------------------------
    a_in = [nc.dram_tensor(f"a_in_c{ci}", [g.n_agrp, P, g.ag, sz], F8E4,
                           kind="ExternalInput")
            for ci, (off, sz) in enumerate(g.chunks)]
    g0_in = nc.dram_tensor("g0_in", [P, g.nt, g.f], F8E4, kind="ExternalInput")
    xt_in = nc.dram_tensor("xt_in", [P, g.dloc], F32, kind="ExternalInput")
    disp_in = nc.dram_tensor("disp_in", [P, g.dloc], F32, kind="ExternalInput")
    dispt_in = nc.dram_tensor("dispt_in", [P, g.tpc], F32,
                              kind="ExternalInput")
    w1_in = nc.dram_tensor("w1_in", [P, g.k, g.hid], BF16,
                           kind="ExternalInput")
    w2_in = nc.dram_tensor("w2_in", [P, g.k, g.c], BF16, kind="ExternalInput")
    bb_in = nc.dram_tensor("bb_in", [P, 2], F32, kind="ExternalInput")

    out_dram = nc.dram_tensor("out", [g.dloc, g.c], F32, kind="ExternalOutput")

    n_ag = 5  # allgathers: L1 T1, L1 T2, h, L2 T1, L2 T2
    groups = [list(range(NCORES))]

    with tile.TileContext(nc) as tc:
        with (
            tc.tile_pool(name="pers", bufs=1) as pers,
            tc.tile_pool(name="work", bufs=1) as work,
            tc.tile_pool(name="psum", bufs=1, space="PSUM") as psp,
            tc.tile_pool(name="dram", bufs=1, space="DRAM") as drp,
        ):
            # ---- persistent SBUF ---------------------------------------
            a_sb = [[pers.tile([P, g.ag, sz], F8E4, tag=f"a{ci}_{i}",
                                name=f"a{ci}_{i}")
                     for i in range(g.n_agrp)]
                    for ci, (off, sz) in enumerate(g.chunks)]
            gbufC = [[pers.tile([P, g.n_ggrp, t1 - t0, g.f], F8E4,
                                tag=f"g{b}_{ci}", name=f"g{b}_{ci}")
                      for ci, (t0, t1) in enumerate(g.ctiles)]
                     for b in range(2)]
            t_sb = [pers.tile([P, g.dloc], F32, tag=f"t{i}", name=f"t{i}")
                    for i in range(3)]
            disp = pers.tile([P, g.dloc], F32, name="disp")
            dispt = pers.tile([P, g.tpc], F32, name="dispt")
            acc = pers.tile([P, g.dloc], F32, name="acc")
            tbf = pers.tile([P, g.dloc], BF16, name="tbf")
            tb0 = pers.tile([P, g.dloc], BF16, name="tb0")
            tstage = pers.tile([P, g.tpc, g.f], F8E4, name="tstage")
            w1_sb = pers.tile([P, g.k, g.hid], BF16, name="w1_sb")
            w2_sb = pers.tile([P, g.k, g.c], BF16, name="w2_sb")
            bb_sb = pers.tile([P, 2], F32, name="bb_sb")
            idf32 = pers.tile([P, P], F32, name="idf32")
            idbf = pers.tile([P, P], BF16, name="idbf")
            t_sb0 = pers.tile([P, g.dloc], F32, name="xt")
            prime = work.tile([P, 1], F32, name="prime")

            # ---- DRAM bounce buffers for the collectives ---------------
            warm_dst = drp.tile([NCORES * P, 2], F32, addr_space="Shared",
                                name="warm_dst")
            ag_srcC = [[drp.tile([P, (t1 - t0) * g.f], F8E4,
                                 name=f"ag_src{i}_{ci}")
                        for ci, (t0, t1) in enumerate(g.ctiles)]
                       for i in range(n_ag)]
            ag_dstC = [[drp.tile([NCORES * P, (t1 - t0) * g.f], F8E4,
                                 addr_space="Shared", name=f"ag_dst{i}_{ci}")
                        for ci, (t0, t1) in enumerate(g.ctiles)]
                       for i in range(n_ag)]
            make_identity(nc, idf32[:])
            make_identity(nc, idbf[:])

            # ---- dummy collective: starts the CC cold init at t~0 ------
            warm_sb = work.tile([P, 2], F32, name="warm_sb")
            warm_src = drp.tile([P, 2], F32, name="warm_src")
            nc.gpsimd.memset(warm_sb[:], 0.0)
            nc.gpsimd.dma_start(warm_src[:], warm_sb[:])
            nc.gpsimd.collective_compute(
                "AllGather",
                mybir.AluOpType.bypass,
                replica_groups=groups,
                ins=[warm_src[:]],
                outs=[warm_dst[:]],
            )

            # ---- loads, spread over four DGE queues, ordered by first
            # consumption: g0 + A chunk 0 race first (k=1 streams behind
            # them), then the small tensors, then A chunks 1, 2.
            dges = [nc.sync, nc.scalar, nc.gpsimd]
            loads = []
            g0_4d = g0_in.ap().rearrange("p (j t) f -> p j t f", j=g.n_ggrp)
            for ci, (t0, t1) in enumerate(g.ctiles):
                loads.append((gbufC[0][ci], g0_4d[:, :, t0:t1, :]))
            for i in range(g.n_agrp):
                loads.append((a_sb[0][i], a_in[0][i]))
            loads.append((t_sb0, xt_in.ap()))
            loads.append((w1_sb, w1_in.ap()))
            loads.append((w2_sb, w2_in.ap()))
            loads.append((bb_sb, bb_in.ap()))
            loads.append((dispt, dispt_in.ap()))
            loads.append((disp, disp_in.ap()))
            for ci in range(1, len(g.chunks)):
                for i in range(g.n_agrp):
                    loads.append((a_sb[ci][i], a_in[ci][i]))
            for ld, (dst, src) in enumerate(loads):
                dges[ld % len(dges)].dma_start(dst[:], src)

            def warm(n_mm, anchor, off, sz):
                """AG-wait filler matmuls, anchored (rhs=tbf slice) so the
                scheduler can't hoist them to t=0."""
                w = min(sz, 256)
                for _ in range(n_mm):
                    wp = psp.tile([P, 256], F32, space="PSUM", tag="warm",
                                  name="wp")
                    nc.tensor.matmul(wp[:, :w], lhsT=idbf[:],
                                     rhs=anchor[:, off:off + w],
                                     start=True, stop=True,
                                     skip_group_check=True)

            ag_state = {"idx": 0, "cur": 0}

            def chunk_tiles(off, sz):
                return range(off // P, (off + sz) // P)

            def stage_chunk(idx, ci, src_f32, off, sz):
                """transpose fp32 tiles on the PE, fuse dis-scale + fp8 cast
                in the PSUM->SBUF copy, stage to ag_src."""
                for t in chunk_tiles(off, sz):
                    tpb = psp.tile([P, P], F32, space="PSUM", tag="tpb",
                                   name="tpb", bufs=2)
                    nc.tensor.transpose(out=tpb[:],
                                        in_=src_f32[:, t * P:(t + 1) * P],
                                        identity=idf32[:])
                    nc.scalar.mul(tstage[:, t, :], tpb[:], dispt[:, t:t + 1])
                t0, t1 = off // P, (off + sz) // P
                nc.scalar.dma_start(ag_srcC[idx][ci][:], tstage[:, t0:t1, :])

            def allgather_chunk(idx, ci, b_next, last):
                nc.gpsimd.collective_compute(
                    "AllGather",
                    mybir.AluOpType.bypass,
                    replica_groups=groups,
                    ins=[ag_srcC[idx][ci][:]],
                    outs=[ag_dstC[idx][ci][:]],
                )
                if last and idx > 0:
                    off, sz = g.chunks[ci]
                    warm(N_WARM_STEP, tbf, off, sz)
                nc.sync.dma_start(
                    gbufC[b_next][ci][:],
                    ag_dstC[idx][ci][:, :]
                    .rearrange("(j p) (t f) -> p j t f", p=P, f=g.f),
                )

            def w_term_chunk(w_sb, k, rhs_bf, cdim, off, sz):
                """acc[0:cdim, chunk] (+)= (T_k @ W[k])^T, bf16 matmul."""
                wt = psp.tile([P, 512], F32, space="PSUM", tag="wt",
                              name="wt", bufs=2)
                nc.tensor.matmul(
                    wt[:cdim, :sz],
                    lhsT=w_sb[:, k, :],
                    rhs=rhs_bf[:, off:off + sz],
                    start=True, stop=True,
                )
                if k == 0:
                    nc.vector.tensor_copy(acc[:cdim, off:off + sz],
                                          wt[:cdim, :sz])
                else:
                    nc.vector.tensor_add(acc[:cdim, off:off + sz],
                                         acc[:cdim, off:off + sz],
                                         wt[:cdim, :sz])

            z_all = work.tile([P, g.tpc, g.c], F32, name="z_all")
            m_all = work.tile([P, g.tpc, 1], F32, name="m_all")
            e_all = work.tile([P, g.tpc, g.c], F32, name="e_all")
            s_all = work.tile([P, g.tpc, 1], F32, name="s_all")
            o_all = work.tile([P, g.tpc, g.c], F32, name="o_all")
            out_ap = out_dram.ap().rearrange("(t p) c -> p t c", p=P)

            def final_chunk_tail(ci, off, sz):
                """final layer, per chunk: bias (DVE) + transpose to
                node-major + max/sub. The exp/ln tail is batched at the
                end (ACT table loads are 1.28us stalls)."""
                t0, t1 = off // P, (off + sz) // P
                nt = t1 - t0
                nc.vector.tensor_tensor(
                    out=acc[:g.c, off:off + sz],
                    in0=acc[:g.c, off:off + sz],
                    in1=bb_sb[:g.c, 1:2].to_broadcast([g.c, sz]),
                    op=mybir.AluOpType.add)
                for t in chunk_tiles(off, sz):
                    zp = psp.tile([P, g.c], F32, space="PSUM",
                                  tag="tpb", name="zp", bufs=2)
                    nc.tensor.transpose(
                        out=zp[:],
                        in_=acc[:g.c, t * P:(t + 1) * P],
                        identity=idf32[:g.c, :g.c])
                    nc.vector.tensor_copy(z_all[:, t, :], zp[:])
                z = z_all[:, t0:t1, :]
                m = m_all[:, t0:t1, :]
                nc.vector.tensor_reduce(out=m[:, :, 0], in_=z,
                                        axis=mybir.AxisListType.X,
                                        op=mybir.AluOpType.max)
                nc.vector.tensor_tensor(out=e_all[:, t0:t1, :], in0=z,
                                        in1=m.to_broadcast([P, nt, g.c]),
                                        op=mybir.AluOpType.subtract)

            def final_softmax_tail():
                """batched exp/sum/ln/sub over the whole [P, tpc, c]."""
                nc.scalar.activation(o_all[:], e_all[:],
                                     mybir.ActivationFunctionType.Exp)
                nc.vector.tensor_reduce(out=s_all[:, :, 0], in_=o_all[:],
                                        axis=mybir.AxisListType.X,
                                        op=mybir.AluOpType.add)
                nc.scalar.activation(s_all[:], s_all[:],
                                     mybir.ActivationFunctionType.Ln)
                nc.vector.tensor_tensor(
                    out=o_all[:], in0=e_all[:],
                    in1=s_all[:].to_broadcast([P, g.tpc, g.c]),
                    op=mybir.AluOpType.subtract)
                nc.sync.dma_start(out_ap[:, :, :], o_all[:])

            # ---- pair helpers ------------------------------------------
            def pairs_arrival():
                """k=1 layer-0 order: ascending gi == A-group arrival."""
                out = []
                for j in range(g.n_ggrp):
                    for t in range(0, g.tpc, 2):
                        out.append((g.gci_of_tile(t), j, t))
                return out

            def pairs_gci(gci_list):
                return [(gci, j, g.ctiles[gci][0] + 2 * p)
                        for gci in gci_list
                        for j in range(g.n_ggrp)
                        for p in range((g.ctiles[gci][1]
                                        - g.ctiles[gci][0]) // 2)]

            n_pairs = g.nt // 2  # per output chunk

            def emit_pairs(pp, ci, sz, plist, cur, counter):
                """emit DoubleRow matmuls for pairs of output chunk ci."""
                for (gci, j, t) in plist:
                    gi = j * g.tpc + t
                    ts0 = g.ctiles[gci][0]
                    lhs = gbufC[cur][gci][:, j, t - ts0:t - ts0 + 2, :]
                    rhs = a_sb[ci][gi // g.ag][:, gi % g.ag:gi % g.ag + 2, :]
                    nc.tensor.matmul(
                        pp[:, :sz],
                        lhsT=lhs,
                        rhs=rhs,
                        start=(counter[ci] == 0),
                        stop=(counter[ci] == n_pairs - 1),
                        perf_mode=mybir.MatmulPerfMode.DoubleRow,
                    )
                    counter[ci] += 1

            # ---- per-chunk recursion + tail ----------------------------
            def recursion_chunk(pp, layer, k, ci, off, sz):
                tk = t_sb[k % 3]
                if k == 1:
                    nc.vector.scalar_tensor_tensor(
                        out=tk[:, off:off + sz],
                        in0=pp[:, :sz],
                        scalar=-1.0,
                        in1=disp[:, off:off + sz],
                        op0=mybir.AluOpType.mult,
                        op1=mybir.AluOpType.mult)
                else:
                    tk2 = (t_sb[(k - 2) % 3] if k >= 3 else
                           (t_sb0 if layer == 0 else t_sb[0]))
                    nc.vector.scalar_tensor_tensor(
                        out=tk[:, off:off + sz],
                        in0=pp[:, :sz],
                        scalar=-2.0,
                        in1=disp[:, off:off + sz],
                        op0=mybir.AluOpType.mult,
                        op1=mybir.AluOpType.mult)
                    nc.vector.tensor_sub(
                        tk[:, off:off + sz],
                        tk[:, off:off + sz],
                        tk2[:, off:off + sz])

            def tail_chunk(layer, k, ci):
                off, sz = g.chunks[ci]
                tk = t_sb[k % 3]
                w_sb = w1_sb if layer == 0 else w2_sb
                cdim = g.hid if layer == 0 else g.c
                do_stage = k < g.k - 1
                last = (ci == len(g.chunks) - 1)
                if layer == 0 and k == 1:
                    # T0 W-term first (acc base is a copy, must precede adds)
                    if ci == 0:
                        nc.vector.tensor_copy(tb0[:], t_sb0[:])
                    w_term_chunk(w_sb, 0, tb0, cdim, off, sz)
                if do_stage:
                    stage_chunk(ag_state["idx"], ci, tk, off, sz)
                    nc.vector.tensor_copy(tbf[:, off:off + sz],
                                          tk[:, off:off + sz])
                    w_term_chunk(w_sb, k, tbf, cdim, off, sz)
                    allgather_chunk(ag_state["idx"], ci,
                                    1 - ag_state["cur"], last)
                    return
                nc.vector.tensor_copy(tbf[:, off:off + sz],
                                      tk[:, off:off + sz])
                w_term_chunk(w_sb, k, tbf, cdim, off, sz)
                if layer == 0:
                    # layer end: h = relu(acc + b1) once, bf16 mirror on DVE
                    nc.scalar.activation(
                        t_sb[0][:, off:off + sz],
                        acc[:, off:off + sz],
                        mybir.ActivationFunctionType.Relu,
                        bias=bb_sb[:, 0:1], scale=1.0)
                    stage_chunk(ag_state["idx"], ci, t_sb[0], off, sz)
                    nc.vector.tensor_copy(tb0[:, off:off + sz],
                                          t_sb[0][:, off:off + sz])
                    if last:
                        # prime the Exp ACT table during layer-2's slack
                        nc.scalar.activation(
                            prime[:], bb_sb[:, 0:1],
                            mybir.ActivationFunctionType.Exp)
                    allgather_chunk(ag_state["idx"], ci,
                                    1 - ag_state["cur"], last)
                else:
                    final_chunk_tail(ci, off, sz)
                    if last:
                        final_softmax_tail()

            def end_exchange():
                ag_state["idx"] += 1
                ag_state["cur"] = 1 - ag_state["cur"]

            # ---- the two ChebConv layers -------------------------------
            for layer in range(2):
                if layer == 1:
                    # T0 term for layer 2 (tb0 = bf16 h): fills the h-AG wait
                    w_sb, cdim = w2_sb, g.c
                    for (off, sz) in g.chunks:
                        w_term_chunk(w_sb, 0, tb0, cdim, off, sz)
                for k in range(1, g.k):
                    cur = ag_state["cur"]
                    do_stage = k < g.k - 1
                    do_ag = do_stage or layer == 0
                    counter = {ci: 0 for ci in range(len(g.chunks))}
                    pp = {}
                    for ci, (off, sz) in enumerate(g.chunks):
                        pp[ci] = psp.tile([P, 512], F32, space="PSUM",
                                          tag="pp", name=f"pp{ci}", bufs=3)
                    if layer == 0 and k == 1:
                        # A-arrival order, chunk-major
                        for ci, (off, sz) in enumerate(g.chunks):
                            emit_pairs(pp[ci], ci, sz, pairs_arrival(),
                                       cur, counter)
                            recursion_chunk(pp[ci], layer, k, ci, off, sz)
                            tail_chunk(layer, k, ci)
                    elif len(g.chunks) != 3 or len(g.ctiles) != 3:
                        # small/sanity geometries: plain chunk-major
                        all_gci = list(range(len(g.ctiles)))
                        for ci, (off, sz) in enumerate(g.chunks):
                            emit_pairs(pp[ci], ci, sz, pairs_gci(all_gci),
                                       cur, counter)
                            recursion_chunk(pp[ci], layer, k, ci, off, sz)
                            tail_chunk(layer, k, ci)
                    else:
                        # hybrid: c0 fully first (early AG), then gci-major
                        # with stages emitted one sub-section late so the
                        # PE never stalls on the DVE recursion.
                        c0, c1, c2 = 0, 1, 2
                        emit_pairs(pp[c0], c0, g.chunks[c0][1],
                                   pairs_gci([0, 1, 2]), cur, counter)
                        recursion_chunk(pp[c0], layer, k, c0,
                                        *g.chunks[c0])
                        emit_pairs(pp[c1], c1, g.chunks[c1][1],
                                   pairs_gci([0]), cur, counter)
                        tail_chunk(layer, k, c0)
                        emit_pairs(pp[c2], c2, g.chunks[c2][1],
                                   pairs_gci([0]), cur, counter)
                        emit_pairs(pp[c1], c1, g.chunks[c1][1],
                                   pairs_gci([1, 2]), cur, counter)
                        recursion_chunk(pp[c1], layer, k, c1,
                                        *g.chunks[c1])
                        emit_pairs(pp[c2], c2, g.chunks[c2][1],
                                   pairs_gci([1]), cur, counter)
                        tail_chunk(layer, k, c1)
                        emit_pairs(pp[c2], c2, g.chunks[c2][1],
                                   pairs_gci([2]), cur, counter)
                        recursion_chunk(pp[c2], layer, k, c2,
                                        *g.chunks[c2])
                        tail_chunk(layer, k, c2)
                    assert all(counter[ci] == n_pairs for ci in counter)
                    if do_ag:
                        end_exchange()

    nc.compile()
    return nc


def host_prep(g: Geom, x, edge_index, W1, b1, W2, b2):
    """Build the per-core input maps (sharding + dense-ification)."""
    n = g.n
    src = np.asarray(edge_index[0], dtype=np.int64)
    dst = np.asarray(edge_index[1], dtype=np.int64)
    deg = np.bincount(src, minlength=n).astype(np.float64)
    dis = np.where(deg > 0, 1.0 / np.sqrt(np.maximum(deg, 1e-12)), 0.0)

    cnt_t = np.zeros((g.npad, g.npad), dtype=np.float32)
    np.add.at(cnt_t, (src, dst), 1.0)

    dis_pad = np.zeros(g.npad, dtype=np.float32)
    dis_pad[:n] = dis.astype(np.float32)
    x_pad = np.zeros((g.npad, g.f), dtype=np.float32)
    x_pad[:n] = np.asarray(x, dtype=np.float32)

    g0 = dis_pad[:, None] * x_pad  # [npad, f]
    g0_tiles = (g0.reshape(g.nt, P, g.f).transpose(1, 0, 2)
                .astype(ml_dtypes.float8_e4m3))  # [128, nt, f]

    w1 = np.ascontiguousarray(
        np.asarray(W1, np.float32).transpose(1, 0, 2)
    ).astype(ml_dtypes.bfloat16)  # [P, k, hid]
    w2 = np.ascontiguousarray(
        np.asarray(W2, np.float32).transpose(1, 0, 2)
    ).astype(ml_dtypes.bfloat16)  # [P, k, c]
    bb = np.zeros((P, 2), np.float32)
    bb[:g.hid, 0] = np.asarray(b1, np.float32)
    bb[:g.c, 1] = np.asarray(b2, np.float32)

    in_maps = []
    for c in range(NCORES):
        lo, hi = c * g.dloc, (c + 1) * g.dloc
        a_c = (cnt_t[:, lo:hi].astype(ml_dtypes.float8_e4m3)
               .reshape(g.n_agrp, g.ag, P, g.dloc).transpose(0, 2, 1, 3))
        a_chunks = [np.ascontiguousarray(a_c[:, :, :, off:off + sz])
                    for (off, sz) in g.chunks]
        xt = np.ascontiguousarray(x_pad[lo:hi].T)          # [128, dloc]
        d_loc = dis_pad[lo:hi]
        disp = np.ascontiguousarray(
            np.broadcast_to(d_loc[None, :], (P, g.dloc))).astype(np.float32)
        dispt = np.ascontiguousarray(
            d_loc.reshape(g.tpc, P).T).astype(np.float32)  # [128, tpc]
        im = {f"a_in_c{ci}": a_chunks[ci] for ci in range(len(g.chunks))}
        im.update({
            "g0_in": np.ascontiguousarray(g0_tiles),
            "xt_in": xt,
            "disp_in": disp,
            "dispt_in": dispt,
            "w1_in": w1,
            "w2_in": w2,
            "bb_in": bb,
        })
        in_maps.append(im)
    return in_maps


_CACHED_NC = None


def _get_nc():
    global _CACHED_NC
    if _CACHED_NC is None:
        _CACHED_NC = build_nc(FULL)
    return _CACHED_NC


def _enable_ldw_opt():
    """The default axon compile flags pass --enable-ldw-opt=false, which
    serializes every LDWEIGHTS with its MATMUL (~+107ns per matmul). Our
    kernel is a long stream of ldweights+matmul pairs, so re-enable it."""
    try:
        from concourse.compiler_utils import (get_compiler_flags,
                                              set_compiler_flags)
        flags = get_compiler_flags()
        new = [f.replace("--enable-ldw-opt=false", "--enable-ldw-opt=true")
               for f in flags]
        if new != flags:
            set_compiler_flags(new)
    except Exception:
        pass


def kernel(x, edge_index, W1, b1, W2, b2, _profile=False):
    g = FULL
    _enable_ldw_opt()
    in_maps = host_prep(g, x, edge_index, W1, b1, W2, b2)
    nc = _get_nc()
    res = run_bass_kernel_spmd(nc, in_maps, list(range(NCORES)),
                               trace=_profile)
    out = np.concatenate([res.results[c]["out"] for c in range(NCORES)], 0)
    out = out[:g.n].astype(np.float32)
    if _profile:
        kernel.last_result = res
    return out


# revision 13
# speedup vs baseline: 1.0586x; 1.0586x over previous
"""ChebNet (K=4, 2 ChebConv layers + relu + log_softmax) on 8 trn2 NeuronCores.

Strategy (graph/data parallel, dense-ified SpMM + host-precomputed Chebyshev
polynomial matrices):
  - prop matrices act on g = fp8(dis * value):
      T1 = A_hat x        = -diag(dis) @ Cnt^T @ g          (Cnt exact in fp8)
      T2 = (2A^2 - I) x   = desc2 * (M2 @ g) - x,   M2 = 2A^2 diag(1/dis)
      T3 = (4A^3 - 3A) x  = desc3 * (M3 @ g) - 3*T1, M3 = 4A^3 diag(1/dis)
    M2/M3 are host-precomputed, row-scaled (pow2) and fp8-quantized; the
    -x / -3*T1 corrections happen on-device in fp32, which keeps the
    quantization error at the same level as the pure-recursion design
    (validated 4.4e-3 vs reference).
  - Nodes sharded 8 ways by destination. Cnt^T shard (12.6 MB fp8) stays
    SBUF-resident; M2/M3 shards are STREAMED from HBM through a small
    rotating window (each entry is used once per layer), so the kernel
    needs only ONE collective in total: the AllGather of h between the
    layers. This sidesteps the collective path's variable 60-140us cold
    start and its ~10us/chunk service pace that dominated the
    recursion-based design (5 exchanges, 15 collectives, ~320us).
  - All matmuls are fp8 DoubleRow (256-src contraction / instruction,
    157 TF/s). Layer-1 T1 is emitted in A-arrival order so the PE streams
    right behind the initial HBM load.
  - Tail: biases/casts on DVE, log_softmax batched with the Exp ACT table
    primed early (ACT_TABLE_LOAD is a 1.28us stall), single output DMA.
"""

import sys

sys.path.insert(0, "/opt/trn_rl_repo")

import numpy as np
import ml_dtypes

import concourse.bacc as bacc
import concourse.mybir as mybir
import concourse.tile as tile
from concourse.bass_utils import run_bass_kernel_spmd
from concourse.masks import make_identity

F32 = mybir.dt.float32
BF16 = mybir.dt.bfloat16
F8E4 = mybir.dt.float8e4

NCORES = 8
P = 128

N = 10000
F_IN = 128
HID = 128
C_OUT = 16
K_ORD = 4

N_STREAM_SLOTS = 4


class Geom:
    def __init__(self, n_nodes, tiles_per_core, f_in=F_IN, hid=HID, c_out=C_OUT,
                 k_ord=K_ORD):
        self.n = n_nodes
        self.tpc = tiles_per_core          # src tiles per core (DLOC/128)
        self.dloc = tiles_per_core * P     # nodes per core (padded)
        self.npad = self.dloc * NCORES     # padded node count
        self.nt = self.npad // P           # total src tiles
        self.f = f_in
        self.hid = hid
        self.c = c_out
        self.k = k_ord
        assert self.npad >= n_nodes
        assert f_in == P and hid == P
        # psum chunking of the dloc free dim (max 512 fp32 per bank).
        self.chunks = []
        off = 0
        while off < self.dloc:
            sz = 256 if (off == 0 and self.dloc > 512) else \
                min(512, self.dloc - off)
            self.chunks.append((off, sz))
            off += sz
        self.ctiles = [(off // P, (off + sz) // P) for off, sz in self.chunks]
        assert all((t1 - t0) % 2 == 0 for t0, t1 in self.ctiles)
        # a-tile groups (DMA granularity, also the stream piece size)
        self.ag = next(a for a in (8, 16, 40) if self.nt % a == 0)
        self.n_agrp = self.nt // self.ag
        self.n_ggrp = NCORES

    def gci_of_tile(self, t):
        for gci, (t0, t1) in enumerate(self.ctiles):
            if t0 <= t < t1:
                return gci
        raise AssertionError(t)


FULL = Geom(N, 10)  # 1280 nodes/core, npad=10240, 80 src tiles


def build_nc(g: Geom):
    nc = bacc.Bacc("TRN2", target_bir_lowering=False, debug=False,
                   num_devices=NCORES)

    # ---- kernel I/O ----------------------------------------------------
    a_in = [nc.dram_tensor(f"a_in_c{ci}", [g.n_agrp, P, g.ag, sz], F8E4,
                           kind="ExternalInput")
            for ci, (off, sz) in enumerate(g.chunks)]
    m2_in = [nc.dram_tensor(f"m2_in_c{ci}", [g.n_agrp, P, g.ag, sz], F8E4,
                            kind="ExternalInput")
             for ci, (off, sz) in enumerate(g.chunks)]
    m3_in = [nc.dram_tensor(f"m3_in_c{ci}", [g.n_agrp, P, g.ag, sz], F8E4,
                            kind="ExternalInput")
             for ci, (off, sz) in enumerate(g.chunks)]
    g0_in = nc.dram_tensor("g0_in", [P, g.nt, g.f], F8E4, kind="ExternalInput")
    xt_in = nc.dram_tensor("xt_in", [P, g.dloc], F32, kind="ExternalInput")
    disp_in = nc.dram_tensor("disp_in", [P, g.dloc], F32, kind="ExternalInput")
    desc2_in = nc.dram_tensor("desc2_in", [P, g.dloc], F32,
                              kind="ExternalInput")
    desc3_in = nc.dram_tensor("desc3_in", [P, g.dloc], F32,
                              kind="ExternalInput")
    dispt_in = nc.dram_tensor("dispt_in", [P, g.tpc], F32,
                              kind="ExternalInput")
    w1_in = nc.dram_tensor("w1_in", [P, g.k, g.hid], BF16,
                           kind="ExternalInput")
    w2_in = nc.dram_tensor("w2_in", [P, g.k, g.c], BF16, kind="ExternalInput")
    bb_in = nc.dram_tensor("bb_in", [P, 2], F32, kind="ExternalInput")

    out_dram = nc.dram_tensor("out", [g.dloc, g.c], F32, kind="ExternalOutput")

    groups = [list(range(NCORES))]
    S = N_STREAM_SLOTS

    with tile.TileContext(nc) as tc:
        with (
            tc.tile_pool(name="pers", bufs=1) as pers,
            tc.tile_pool(name="work", bufs=1) as work,
            tc.tile_pool(name="psum", bufs=1, space="PSUM") as psp,
            tc.tile_pool(name="dram", bufs=1, space="DRAM") as drp,
        ):
            # ---- persistent SBUF ---------------------------------------
            a_sb = [[pers.tile([P, g.ag, sz], F8E4, tag=f"a{ci}_{i}",
                                name=f"a{ci}_{i}")
                     for i in range(g.n_agrp)]
                    for ci, (off, sz) in enumerate(g.chunks)]
            # single g buffer set: g0 for layer 1, h (AG result) for layer 2
            gbufC = [pers.tile([P, g.n_ggrp, t1 - t0, g.f], F8E4,
                               tag=f"g_{ci}", name=f"g_{ci}")
                     for ci, (t0, t1) in enumerate(g.ctiles)]
            t_sb = [pers.tile([P, g.dloc], F32, tag=f"t{i}", name=f"t{i}")
                    for i in range(3)]
            disp = pers.tile([P, g.dloc], F32, name="disp")
            desc2 = pers.tile([P, g.dloc], F32, name="desc2")
            desc3 = pers.tile([P, g.dloc], F32, name="desc3")
            dispt = pers.tile([P, g.tpc], F32, name="dispt")
            acc = pers.tile([P, g.dloc], F32, name="acc")
            tbf = pers.tile([P, g.dloc], BF16, name="tbf")
            tb0 = pers.tile([P, g.dloc], BF16, name="tb0")
            tstage = pers.tile([P, g.tpc, g.f], F8E4, name="tstage")
            w1_sb = pers.tile([P, g.k, g.hid], BF16, name="w1_sb")
            w2_sb = pers.tile([P, g.k, g.c], BF16, name="w2_sb")
            bb_sb = pers.tile([P, 2], F32, name="bb_sb")
            idf32 = pers.tile([P, P], F32, name="idf32")
            t_sb0 = pers.tile([P, g.dloc], F32, name="xt")
            prime = work.tile([P, 1], F32, name="prime")

            # ---- DRAM bounce buffers for the single h exchange ---------
            ag_srcC = [drp.tile([P, (t1 - t0) * g.f], F8E4,
                                name=f"ag_src_{ci}")
                       for ci, (t0, t1) in enumerate(g.ctiles)]
            ag_dstC = [drp.tile([NCORES * P, (t1 - t0) * g.f], F8E4,
                                addr_space="Shared", name=f"ag_dst_{ci}")
                       for ci, (t0, t1) in enumerate(g.ctiles)]
            make_identity(nc, idf32[:])

            # ---- initial loads: g0 + A chunk0 race first, then smalls,
            # then A chunks 1,2. bulk on scalar+gpsimd; sync reserved for
            # the latency-critical stage/g-load/out path.
            bulk = [nc.scalar, nc.gpsimd]
            hh = g.ag // 2

            def half_load(dst, src):
                bulk[0].dma_start(dst[:, 0:hh, :], src[:, 0:hh, :])
                bulk[1].dma_start(dst[:, hh:, :], src[:, hh:, :])

            g0_4d = g0_in.ap().rearrange("p (j t) f -> p j t f", j=g.n_ggrp)
            for ci, (t0, t1) in enumerate(g.ctiles):
                bulk[ci % 2].dma_start(gbufC[ci][:], g0_4d[:, :, t0:t1, :])
            for i in range(g.n_agrp):
                half_load(a_sb[0][i], a_in[0][i])
            smalls = [(t_sb0, xt_in), (w1_sb, w1_in), (w2_sb, w2_in),
                      (bb_sb, bb_in), (dispt, dispt_in), (disp, disp_in),
                      (desc2, desc2_in), (desc3, desc3_in)]
            for ld, (dst, src) in enumerate(smalls):
                bulk[ld % 2].dma_start(dst[:], src.ap())
            for ci in range(1, len(g.chunks)):
                for i in range(g.n_agrp):
                    half_load(a_sb[ci][i], a_in[ci][i])

            # rotating stream windows for M2/M3 pieces
            ms_sb = [[pers.tile([P, g.ag, sz], F8E4, tag=f"ms{ci}_{s}",
                                name=f"ms{ci}_{s}")
                      for s in range(S)]
                     for ci, (off, sz) in enumerate(g.chunks)]
            stream_ctr = {"i": 0}

            n_pairs = g.nt // 2  # per output chunk

            def pairs_arrival():
                """ascending gi == src-tile arrival order."""
                return [(j * g.tpc + t)
                        for j in range(g.n_ggrp)
                        for t in range(0, g.tpc, 2)]

            def pairs_gci():
                """AG-chunk arrival order (for layer-2 T1)."""
                return [(j * g.tpc + g.ctiles[gci][0] + 2 * p)
                        for gci in range(len(g.ctiles))
                        for j in range(g.n_ggrp)
                        for p in range((g.ctiles[gci][1]
                                        - g.ctiles[gci][0]) // 2)]

            def lhs_of(gi):
                j, t = gi // g.tpc, gi % g.tpc
                gci = g.gci_of_tile(t)
                ts0 = g.ctiles[gci][0]
                return gbufC[gci][:, j, t - ts0:t - ts0 + 2, :]

            def emit_resident(pp, ci, sz, gi_list):
                """DoubleRow matmuls vs the resident Cnt shard."""
                for n_i, gi in enumerate(gi_list):
                    rhs = a_sb[ci][gi // g.ag][:, gi % g.ag:gi % g.ag + 2, :]
                    nc.tensor.matmul(
                        pp[:, :sz], lhsT=lhs_of(gi), rhs=rhs,
                        start=(n_i == 0), stop=(n_i == n_pairs - 1),
                        perf_mode=mybir.MatmulPerfMode.DoubleRow,
                    )

            def emit_streamed(pp, ci, sz, m_in, into_a=False):
                """DoubleRow matmuls vs streamed pieces of M2/M3. Each piece
                lands as two concurrent half-DMAs (one per bulk queue) so a
                single DGE's issue+transfer serialization can't cap the
                stream rate. Layer-2 re-streams overwrite the dead Cnt shard
                (into_a) for unbounded lookahead."""
                h = g.ag // 2
                for agrp in range(g.n_agrp):
                    if into_a:
                        slot = a_sb[ci][agrp]
                    else:
                        slot = ms_sb[ci][stream_ctr["i"] % S]
                        stream_ctr["i"] += 1
                    piece = m_in[ci][agrp]
                    bulk[0].dma_start(slot[:, 0:h, :], piece[:, 0:h, :])
                    bulk[1].dma_start(slot[:, h:, :], piece[:, h:, :])
                    for p_i in range(g.ag // 2):
                        gi = agrp * g.ag + 2 * p_i
                        n_i = agrp * (g.ag // 2) + p_i
                        nc.tensor.matmul(
                            pp[:, :sz], lhsT=lhs_of(gi),
                            rhs=slot[:, 2 * p_i:2 * p_i + 2, :],
                            start=(n_i == 0), stop=(n_i == n_pairs - 1),
                            perf_mode=mybir.MatmulPerfMode.DoubleRow,
                        )

            def w_term_chunk(w_sb, k, rhs_bf, cdim, off, sz):
                """acc[0:cdim, chunk] (+)= (T_k @ W[k])^T, bf16 matmul."""
                wt = psp.tile([P, 512], F32, space="PSUM", tag="wt",
                              name="wt", bufs=2)
                nc.tensor.matmul(
                    wt[:cdim, :sz],
                    lhsT=w_sb[:, k, :],
                    rhs=rhs_bf[:, off:off + sz],
                    start=True, stop=True,
                )
                if k == 0:
                    nc.vector.tensor_copy(acc[:cdim, off:off + sz],
                                          wt[:cdim, :sz])
                else:
                    nc.vector.tensor_add(acc[:cdim, off:off + sz],
                                         acc[:cdim, off:off + sz],
                                         wt[:cdim, :sz])

            def chunk_tiles(off, sz):
                return range(off // P, (off + sz) // P)

            def stage_chunk(ci, src_f32, off, sz):
                """transpose fp32 tiles on the PE, fuse dis-scale + fp8 cast
                in the PSUM->SBUF copy, stage to ag_src."""
                for t in chunk_tiles(off, sz):
                    tpb = psp.tile([P, P], F32, space="PSUM", tag="tpb",
                                   name="tpb", bufs=2)
                    nc.tensor.transpose(out=tpb[:],
                                        in_=src_f32[:, t * P:(t + 1) * P],
                                        identity=idf32[:])
                    nc.scalar.mul(tstage[:, t, :], tpb[:], dispt[:, t:t + 1])
                t0, t1 = off // P, (off + sz) // P
                nc.sync.dma_start(ag_srcC[ci][:], tstage[:, t0:t1, :])

            def allgather_chunk(ci):
                nc.gpsimd.collective_compute(
                    "AllGather",
                    mybir.AluOpType.bypass,
                    replica_groups=groups,
                    ins=[ag_srcC[ci][:]],
                    outs=[ag_dstC[ci][:]],
                )

            def gload_chunk(ci):
                # deferred: gbufC still holds g0 until ALL layer-1 reads
                # (every T3 output chunk contracts over every gci) are done.
                nc.sync.dma_start(
                    gbufC[ci][:],
                    ag_dstC[ci][:, :]
                    .rearrange("(j p) (t f) -> p j t f", p=P, f=g.f),
                )

            z_all = work.tile([P, g.tpc, g.c], F32, name="z_all")
            m_all = work.tile([P, g.tpc, 1], F32, name="m_all")
            e_all = work.tile([P, g.tpc, g.c], F32, name="e_all")
            s_all = work.tile([P, g.tpc, 1], F32, name="s_all")
            o_all = work.tile([P, g.tpc, g.c], F32, name="o_all")
            out_ap = out_dram.ap().rearrange("(t p) c -> p t c", p=P)

            def final_chunk_tail(ci, off, sz):
                t0, t1 = off // P, (off + sz) // P
                nt = t1 - t0
                nc.vector.tensor_tensor(
                    out=acc[:g.c, off:off + sz],
                    in0=acc[:g.c, off:off + sz],
                    in1=bb_sb[:g.c, 1:2].to_broadcast([g.c, sz]),
                    op=mybir.AluOpType.add)
                for t in chunk_tiles(off, sz):
                    zp = psp.tile([P, g.c], F32, space="PSUM",
                                  tag="tpb", name="zp", bufs=2)
                    nc.tensor.transpose(
                        out=zp[:],
                        in_=acc[:g.c, t * P:(t + 1) * P],
                        identity=idf32[:g.c, :g.c])
                    nc.vector.tensor_copy(z_all[:, t, :], zp[:])
                z = z_all[:, t0:t1, :]
                m = m_all[:, t0:t1, :]
                nc.vector.tensor_reduce(out=m[:, :, 0], in_=z,
                                        axis=mybir.AxisListType.X,
                                        op=mybir.AluOpType.max)
                nc.vector.tensor_tensor(out=e_all[:, t0:t1, :], in0=z,
                                        in1=m.to_broadcast([P, nt, g.c]),
                                        op=mybir.AluOpType.subtract)

            def final_softmax_tail():
                nc.scalar.activation(o_all[:], e_all[:],
                                     mybir.ActivationFunctionType.Exp)
                nc.vector.tensor_reduce(out=s_all[:, :, 0], in_=o_all[:],
                                        axis=mybir.AxisListType.X,
                                        op=mybir.AluOpType.add)
                nc.scalar.activation(s_all[:], s_all[:],
                                     mybir.ActivationFunctionType.Ln)
                nc.vector.tensor_tensor(
                    out=o_all[:], in0=e_all[:],
                    in1=s_all[:].to_broadcast([P, g.tpc, g.c]),
                    op=mybir.AluOpType.subtract)
                nc.sync.dma_start(out_ap[:, :, :], o_all[:])

            def stt(out_t, sl, pp, sz, scalar, in1_t):
                nc.vector.scalar_tensor_tensor(
                    out=out_t[:, sl], in0=pp[:, :sz], scalar=scalar,
                    in1=in1_t[:, sl],
                    op0=mybir.AluOpType.mult, op1=mybir.AluOpType.mult)

            # ---- the two ChebConv layers -------------------------------
            # t_sb roles: [0] = U scratch then h, [1] = T1, [2] = T2 then T3
            for layer in range(2):
                w_sb = w1_sb if layer == 0 else w2_sb
                cdim = g.hid if layer == 0 else g.c
                last_ci = len(g.chunks) - 1

                # T0 W-term (fills the A-load / h-AG wait)
                if layer == 0:
                    nc.vector.tensor_copy(tb0[:], t_sb0[:])
                for (off, sz) in g.chunks:
                    w_term_chunk(w_sb, 0, tb0, cdim, off, sz)

                # ---- T1 phase (resident Cnt) ---------------------------
                order = pairs_arrival() if layer == 0 else pairs_gci()
                for ci, (off, sz) in enumerate(g.chunks):
                    sl = slice(off, off + sz)
                    pp = psp.tile([P, 512], F32, space="PSUM", tag="pp",
                                  name=f"ppt1_{ci}", bufs=3)
                    emit_resident(pp, ci, sz, order)
                    stt(t_sb[1], sl, pp, sz, -1.0, disp)
                    nc.vector.tensor_copy(tbf[:, sl], t_sb[1][:, sl])
                    w_term_chunk(w_sb, 1, tbf, cdim, off, sz)

                # ---- T2 phase (streamed M2): T2 = desc2*pp - T0 --------
                t0_t = t_sb0 if layer == 0 else t_sb[0]
                for ci, (off, sz) in enumerate(g.chunks):
                    sl = slice(off, off + sz)
                    pp = psp.tile([P, 512], F32, space="PSUM", tag="pp",
                                  name=f"ppt2_{ci}", bufs=3)
                    emit_streamed(pp, ci, sz, m2_in, into_a=(layer == 1))
                    stt(t_sb[2], sl, pp, sz, 1.0, desc2)
                    nc.vector.tensor_sub(t_sb[2][:, sl], t_sb[2][:, sl],
                                         t0_t[:, sl])
                    nc.vector.tensor_copy(tbf[:, sl], t_sb[2][:, sl])
                    w_term_chunk(w_sb, 2, tbf, cdim, off, sz)

                # ---- T3 phase (streamed M3): T3 = desc3*pp - 3*T1 ------
                for ci, (off, sz) in enumerate(g.chunks):
                    sl = slice(off, off + sz)
                    pp = psp.tile([P, 512], F32, space="PSUM", tag="pp",
                                  name=f"ppt3_{ci}", bufs=3)
                    emit_streamed(pp, ci, sz, m3_in, into_a=(layer == 1))
                    stt(t_sb[0], sl, pp, sz, 1.0, desc3)   # U
                    nc.vector.scalar_tensor_tensor(
                        out=t_sb[2][:, sl], in0=t_sb[1][:, sl], scalar=-3.0,
                        in1=t_sb[0][:, sl],
                        op0=mybir.AluOpType.mult, op1=mybir.AluOpType.add)
                    nc.vector.tensor_copy(tbf[:, sl], t_sb[2][:, sl])
                    w_term_chunk(w_sb, 3, tbf, cdim, off, sz)
                    if layer == 0:
                        # h = relu(acc + b1); stage fp8(dis*h); exchange
                        nc.scalar.activation(
                            t_sb[0][:, sl], acc[:, sl],
                            mybir.ActivationFunctionType.Relu,
                            bias=bb_sb[:, 0:1], scale=1.0)
                        stage_chunk(ci, t_sb[0], off, sz)
                        nc.vector.tensor_copy(tb0[:, sl], t_sb[0][:, sl])
                        if ci == last_ci:
                            # prime the Exp ACT table during layer-2 slack
                            nc.scalar.activation(
                                prime[:], bb_sb[:, 0:1],
                                mybir.ActivationFunctionType.Exp)
                        allgather_chunk(ci)
                        if ci == last_ci:
                            for cj in range(len(g.chunks)):
                                gload_chunk(cj)
                    else:
                        final_chunk_tail(ci, off, sz)
                        if ci == last_ci:
                            final_softmax_tail()

    nc.compile()
    return nc


def host_prep(g: Geom, x, edge_index, W1, b1, W2, b2):
    """Shard + dense-ify + precompute the Chebyshev polynomial matrices."""
    import scipy.sparse as sp
    n = g.n
    src = np.asarray(edge_index[0], dtype=np.int64)
    dst = np.asarray(edge_index[1], dtype=np.int64)
    deg = np.bincount(src, minlength=n).astype(np.float64)
    dis = np.where(deg > 0, 1.0 / np.sqrt(np.maximum(deg, 1e-12)), 0.0)

    dis_pad = np.zeros(g.npad, dtype=np.float64)
    dis_pad[:n] = dis
    x_pad = np.zeros((g.npad, g.f), dtype=np.float32)
    x_pad[:n] = np.asarray(x, dtype=np.float32)

    w = np.ones(src.shape[0])
    Csp = sp.csr_matrix((w, (dst, src)), shape=(g.npad, g.npad))
    Ah = (sp.diags(-dis_pad) @ Csp @ sp.diags(dis_pad)).tocsr()
    A2 = (Ah @ Ah).toarray().astype(np.float32)
    A3 = (Ah @ A2).astype(np.float32)

    inv_dis = np.where(dis_pad > 0, 1.0 / np.maximum(dis_pad, 1e-12), 0.0
                       ).astype(np.float32)
    M2 = (2.0 * A2) * inv_dis[None, :]
    M3 = (4.0 * A3) * inv_dis[None, :]
    del A2, A3

    def rowquant(M, target=200.0):
        mx = np.abs(M).max(1)
        mx = np.maximum(mx, 1e-30)
        s = np.exp2(np.floor(np.log2(target / mx))).astype(np.float32)
        Mq = (M * s[:, None]).astype(ml_dtypes.float8_e4m3)
        desc = (1.0 / s).astype(np.float32)
        return Mq, desc

    M2q, d2 = rowquant(M2)
    del M2
    M3q, d3 = rowquant(M3)
    del M3

    # dense-ified edge-count matrix, transposed: cnt_t[s, d]
    cnt_t = np.zeros((g.npad, g.npad), dtype=np.float32)
    np.add.at(cnt_t, (src, dst), 1.0)
    cnt_q = cnt_t.astype(ml_dtypes.float8_e4m3)
    del cnt_t

    g0 = (dis_pad[:, None] * x_pad).astype(np.float32)
    g0_tiles = (g0.reshape(g.nt, P, g.f).transpose(1, 0, 2)
                .astype(ml_dtypes.float8_e4m3))  # [128, nt, f]

    w1 = np.ascontiguousarray(
        np.asarray(W1, np.float32).transpose(1, 0, 2)
    ).astype(ml_dtypes.bfloat16)  # [P, k, hid]
    w2 = np.ascontiguousarray(
        np.asarray(W2, np.float32).transpose(1, 0, 2)
    ).astype(ml_dtypes.bfloat16)  # [P, k, c]
    bb = np.zeros((P, 2), np.float32)
    bb[:g.hid, 0] = np.asarray(b1, np.float32)
    bb[:g.c, 1] = np.asarray(b2, np.float32)

    def shard_mat(Mq_srcdst, lo, hi):
        """[src, dst] fp8 -> per-chunk [n_agrp, P, ag, sz]."""
        mc = (Mq_srcdst[:, lo:hi]
              .reshape(g.n_agrp, g.ag, P, g.dloc).transpose(0, 2, 1, 3))
        return [np.ascontiguousarray(mc[:, :, :, off:off + sz])
                for (off, sz) in g.chunks]

    # M2q/M3q are [dst, src]; device wants [src, dst_local]
    M2qT = np.ascontiguousarray(M2q.T)
    M3qT = np.ascontiguousarray(M3q.T)

    in_maps = []
    for c in range(NCORES):
        lo, hi = c * g.dloc, (c + 1) * g.dloc
        a_chunks = shard_mat(cnt_q, lo, hi)
        m2_chunks = shard_mat(M2qT, lo, hi)
        m3_chunks = shard_mat(M3qT, lo, hi)
        xt = np.ascontiguousarray(x_pad[lo:hi].T)          # [128, dloc]
        d_loc = dis_pad[lo:hi].astype(np.float32)
        disp = np.ascontiguousarray(
            np.broadcast_to(d_loc[None, :], (P, g.dloc))).astype(np.float32)
        desc2b = np.ascontiguousarray(np.broadcast_to(
            d2[lo:hi][None, :], (P, g.dloc))).astype(np.float32)
        desc3b = np.ascontiguousarray(np.broadcast_to(
            d3[lo:hi][None, :], (P, g.dloc))).astype(np.float32)
        dispt = np.ascontiguousarray(
            d_loc.reshape(g.tpc, P).T).astype(np.float32)  # [128, tpc]
        im = {f"a_in_c{ci}": a_chunks[ci] for ci in range(len(g.chunks))}
        im.update({f"m2_in_c{ci}": m2_chunks[ci]
                   for ci in range(len(g.chunks))})
        im.update({f"m3_in_c{ci}": m3_chunks[ci]
                   for ci in range(len(g.chunks))})
        im.update({
            "g0_in": np.ascontiguousarray(g0_tiles),
            "xt_in": xt,
            "disp_in": disp,
            "desc2_in": desc2b,
            "desc3_in": desc3b,
            "dispt_in": dispt,
            "w1_in": w1,
            "w2_in": w2,
            "bb_in": bb,
        })
        in_maps.append(im)
    return in_maps


_CACHED_NC = None


def _get_nc():
    global _CACHED_NC
    if _CACHED_NC is None:
        _CACHED_NC = build_nc(FULL)
    return _CACHED_NC


def _enable_ldw_opt():
    """The default axon compile flags pass --enable-ldw-opt=false, which
    serializes every LDWEIGHTS with its MATMUL (~+107ns per matmul). Our
    kernel is a long stream of ldweights+matmul pairs, so re-enable it."""
    try:
        from concourse.compiler_utils import (get_compiler_flags,
                                              set_compiler_flags)
        flags = get_compiler_flags()
        new = [f.replace("--enable-ldw-opt=false", "--enable-ldw-opt=true")
               for f in flags]
        if new != flags:
            set_compiler_flags(new)
    except Exception:
        pass


def kernel(x, edge_index, W1, b1, W2, b2, _profile=False):
    g = FULL
    _enable_ldw_opt()
    in_maps = host_prep(g, x, edge_index, W1, b1, W2, b2)
    nc = _get_nc()
    res = run_bass_kernel_spmd(nc, in_maps, list(range(NCORES)),
                               trace=_profile)
    out = np.concatenate([res.results[c]["out"] for c in range(NCORES)], 0)
    out = out[:g.n].astype(np.float32)
    if _profile:
        kernel.last_result = res
    return out
